# revision 31
# baseline (speedup 1.0000x reference)
"""Multi-head self-attention Trainium2 kernel (8-core SPMD), v2.

Problem: B=4, S=2048, E=1024, 16 heads x 64 dim, int mask, softmax attention.

Sharding: core c handles batch b=c//2 and head-half hh=c%2 (8 heads).
Each core computes Yp = Attn(X[b])[:, heads(hh)] @ wO[rows(hh)]  -> [S, E]
partial product; host sums the two partials per batch and adds bO.

v2 structure (vs v1): the ACT engine's softmax exp is the spine
(256 instrs x ~1 us); all other work is interleaved under it by emitting
instructions in a single flat "step" schedule (one step per (qh, pair,
k-tile)):
  - only the dd0 projections and two V k-tiles run before attention
    starts; the remaining V k-tiles and QT/KT dd1-3 chunks are emitted
    incrementally during the first pairs' k-loops (all phase-1 PSUM
    groups ride the "sc" slots so they can never deadlock against the
    PV accumulators);
  - scores matmuls are software-pipelined one k-tile ahead of
    exp/mask/PV so the PE never head-of-line blocks the ACT spine;
  - QT/KT PSUM evacuation uses DVE tensor_scalar_add (bias add), V bias
    via DVE tensor_add: ACT does exp only;
  - phase-3 (Y = out^T.T @ wO) groups for q-half 0 are interleaved into
    q-half 1's steps; only half 1's 16 groups remain as tail;
  - normalization keeps the DRAM round-trip broadcast of the reciprocal
    rowsums (latency-tolerant; otn is only needed at phase 3) but uses
    the single-instruction reciprocal_approx_fast custom DVE op;
  - the mask^T tile only holds the current q-half (reloaded per half)
    to fit SBUF alongside the still-live phase-1 tiles.
Layouts are identical to v1: QT/KT [d_all, S] bf16, V stored [V_h|1]
per head (rowsum rides the PV matmul), scores computed transposed
S^T[k,q] per head with the two heads of a pair on PE row-tile halves.
"""

import sys

if "/opt/trn_rl_repo" not in sys.path:
    sys.path.insert(0, "/opt/trn_rl_repo")

import numpy as np
import ml_dtypes

import concourse.bass as bass
import concourse.tile as tile
from concourse import bacc, mybir
from concourse.bass_utils import run_bass_kernel_spmd

F32 = mybir.dt.float32
BF16 = mybir.dt.bfloat16
AF = mybir.ActivationFunctionType

S = 2048      # sequence length
E = 1024      # embed dim
DH = 512      # d_all per core (8 heads x 64)
D = 64        # head dim
H = 8         # heads per core
NE = 8        # embed 128-tiles
ND = 4        # d_all 128-tiles (= head pairs)
NS = 16       # seq 128-tiles
NK = 16       # k 128-tiles
V1W = D + 1   # V columns per head incl. ones column
QH = 1024     # q-half width


def _emit(nc, tc, ctx, d):
    P = 128
    glob = ctx.enter_context(tc.tile_pool(name="glob", bufs=1))

    qt = glob.tile([P, ND * S], BF16)    # QT: [r, p*2048+q], d_all = 128p+r
    kt = glob.tile([P, ND * S], BF16)
    v1 = glob.tile([P, NS * H * V1W], BF16)  # V1: [s%128, st*520 + h*65 + j]
    mt = glob.tile([P, NK * QH], BF16)   # mask^T, current q-half: [r, k*1024+q]
    otn = glob.tile([P, ND * S], BF16)   # normalized out^T
    wo = glob.tile([P, ND * E], BF16)    # wO: [r, p*1024+c], d_all = 128p+r
    bq = glob.tile([P, ND], F32)
    bk = glob.tile([P, ND], F32)
    bvb = glob.tile([P, DH], F32)        # bV broadcast across partitions

    # PSUM: "sc" slots (banks 0-3): QT/KT/V projection groups, score
    # tiles, phase-3 Y tiles; "pv" slots (banks 4-7): PV accumulators
    # ONLY (sharing them with projection groups would deadlock the
    # k-loop against the pair-long accumulator lifetime).
    ps_sc = ctx.enter_context(tc.tile_pool(name="ps_sc", bufs=2, space="PSUM"))
    ps_pv = ctx.enter_context(tc.tile_pool(name="ps_pv", bufs=2, space="PSUM"))

    p1sb = ctx.enter_context(tc.tile_pool(name="p1sb", bufs=1))
    p2str = ctx.enter_context(tc.tile_pool(name="p2str", bufs=5))
    p2nrm = ctx.enter_context(tc.tile_pool(name="p2nrm", bufs=1))
    p2dram = ctx.enter_context(tc.tile_pool(name="p2dram", bufs=2, space="DRAM"))
    p3sb = ctx.enter_context(tc.tile_pool(name="p3sb", bufs=4))

    xt = p1sb.tile([P, NE * S], BF16)    # X^T: [r, e*2048+s], embed = 128e+r
    wq = p1sb.tile([P, NE * DH], BF16)   # wQ: [r, e*512+c]
    wk = p1sb.tile([P, NE * DH], BF16)
    wv = p1sb.tile([P, NE * DH], BF16)

    # ---------------- initial DMAs ----------------
    # wK dd0 + X^T first: they gate the first projection matmuls.
    wk3 = wk[:].rearrange("p (e c) -> p e c", c=DH)
    wksrc = d["wK"].ap().rearrange("(e p) c -> p e c", p=P)
    nc.sync.dma_start(wk3[:, :, 0:P], wksrc[:, :, 0:P])
    for e in range(NE):
        nc.sync.dma_start(
            xt[:, e * S:(e + 1) * S],
            d["XT"].ap().rearrange("(e p) s -> e p s", p=P)[e],
        )
    nc.sync.dma_start(wk3[:, :, P:DH], wksrc[:, :, P:DH])
    nc.sync.dma_start(bk[:], d["bK"].ap().rearrange("(n p) -> p n", p=P))
    nc.sync.dma_start(bq[:], d["bQ"].ap().rearrange("(n p) -> p n", p=P))
    for nm, t in (("wQ", wq), ("wV", wv)):
        nc.sync.dma_start(
            t[:].rearrange("p (e c) -> p e c", c=DH),
            d[nm].ap().rearrange("(e p) c -> p e c", p=P),
        )
    nc.sync.dma_start(
        bvb[:], d["bV"].ap().rearrange("(a s) -> a s", a=1).partition_broadcast(P)
    )
    nc.sync.dma_start(
        wo[:].rearrange("p (n c) -> p n c", c=E),
        d["wO"].ap().rearrange("(n p) c -> p n c", p=P),
    )
    # ones columns of V1 (before V writes; disjoint columns)
    nc.vector.memset(
        v1[:].rearrange("p (t h j) -> p t h j", t=NS, j=V1W)[:, :, :, D:D + 1],
        1.0,
    )


    def emit_mt_dma(k, qh):
        nc.sync.dma_start(
            mt[:, k * QH:(k + 1) * QH],
            d["maskT"].ap()[k * P:(k + 1) * P, qh * QH:(qh + 1) * QH],
        )

    # ---------------- phase-1 chunk emitters (all on "sc" slots) -----
    def emit_kq_chunk(dd, w_sb, out_t, b_t, sc):
        # one 512-wide seq chunk of QT/KT for d_all tile dd
        ps = ps_sc.tile([P, 512], F32, tag="sc", name="kqps")
        for e in range(NE):
            nc.tensor.matmul(
                ps[:],
                w_sb[:, e * DH + dd * P: e * DH + (dd + 1) * P],
                xt[:, e * S + sc * 512: e * S + sc * 512 + 512],
                start=(e == 0), stop=(e == NE - 1),
            )
        nc.vector.tensor_scalar_add(
            out_t[:, dd * S + sc * 512: dd * S + sc * 512 + 512],
            ps[:], b_t[:, dd:dd + 1],
        )

    def emit_v(st):
        # V k-tile: V[s, c] = sum_e XT[e, s] * wV[e, c], bias-add into v1
        ps = ps_sc.tile([P, 512], F32, tag="sc", name="vps")
        for e in range(NE):
            nc.tensor.matmul(
                ps[:],
                xt[:, e * S + st * P: e * S + (st + 1) * P],
                wv[:, e * DH:(e + 1) * DH],
                start=(e == 0), stop=(e == NE - 1),
            )
        dst = v1[:, st * H * V1W:(st + 1) * H * V1W].rearrange(
            "p (h j) -> p h j", j=V1W
        )[:, :, 0:D]
        nc.vector.tensor_add(
            dst,
            ps[:].rearrange("p (h j) -> p h j", j=D),
            bvb[:].rearrange("p (h j) -> p h j", j=D),
        )

    # ---------------- phase-2 per-step pieces ----------------
    cur = {}  # in-flight score tiles keyed by (p, k); "pv" = accumulators

    def emit_scores(qh, p, k):
        qbase = p * S + qh * QH
        s1 = ps_sc.tile([P, 1024], F32, tag="sc", name="s1")
        s2 = ps_sc.tile([P, 1024], F32, tag="sc", name="s2")
        for h, sps in ((0, s1), (1, s2)):
            lo = h * D
            hi = lo + D
            for c in range(2):
                nc.tensor.matmul(
                    sps[:, c * 512:(c + 1) * 512],
                    kt[lo:hi, p * S + k * P: p * S + (k + 1) * P],
                    qt[lo:hi, qbase + c * 512: qbase + c * 512 + 512],
                    start=True, stop=True,
                )
        cur[(p, k)] = (s1, s2)

    def emit_em(qh, p, k):
        # exp -> mask multiply for k-tile k of pair p. MUST be emitted
        # before the next k-tile's scores: those reuse the "sc" slots
        # and slot reuse only orders against already-emitted readers.
        s1, s2 = cur.pop((p, k))
        e1 = p2str.tile([P, 1024], BF16, tag="es", name="e1")
        e2 = p2str.tile([P, 1024], BF16, tag="es", name="e2")
        nc.scalar.activation(e1[:], s1[:], AF.Exp)
        nc.scalar.activation(e2[:], s2[:], AF.Exp)
        pr1 = p2str.tile([P, 1024], BF16, tag="pr", name="pr1")
        pr2 = p2str.tile([P, 1024], BF16, tag="pr", name="pr2")
        mv = mt[:, k * QH:(k + 1) * QH]
        nc.vector.tensor_mul(pr1[:], e1[:], mv)
        nc.vector.tensor_mul(pr2[:], e2[:], mv)
        cur[("pr", p, k)] = (pr1, pr2)

    def emit_pv(qh, p, k):
        # PV accumulate for k-tile k; allocates this pair's accumulators
        # at k==0 (after the previous pair's last PV MMs are emitted, so
        # the pv-slot reuse sees them).
        if k == 0:
            pv1 = ps_pv.tile([V1W, 1024], F32, tag="pv", name="pv1")
            pv2 = ps_pv.tile([V1W, 1024], F32, tag="pv", name="pv2")
            cur["pv"] = (pv1, pv2)
        pv1, pv2 = cur["pv"]
        pr1, pr2 = cur.pop(("pr", p, k))
        for h, pv, pr in ((0, pv1, pr1), (1, pv2, pr2)):
            head = 2 * p + h
            for c in range(2):
                nc.tensor.matmul(
                    pv[:, c * 512:(c + 1) * 512],
                    v1[:, k * H * V1W + head * V1W:
                          k * H * V1W + head * V1W + V1W],
                    pr[:, c * 512:(c + 1) * 512],
                    start=(k == 0), stop=(k == NK - 1),
                )
        return pv1, pv2

    def emit_recips(pv1, pv2):
        # Rowsum reciprocals. nc.vector.reciprocal runs ~6 cycles/elem on
        # a single lane, so 1/x on the [1,1024] rowsum rows directly is
        # ~6.4us each. Instead: rowsums -> DRAM, back as [64,32] (64
        # lanes), reciprocal there (~0.2us), out to DRAM for the
        # partition broadcast in emit_norm_tail. Latency is hidden: otn
        # isn't consumed until phase 3.
        rs = p2nrm.tile([P, 2 * 1024], F32, tag="rs", name="rs")
        nc.vector.tensor_copy(rs[D:D + 1, 0:1024], pv1[D:D + 1, :])
        nc.vector.tensor_copy(rs[D:D + 1, 1024:2048], pv2[D:D + 1, :])
        dr = p2dram.tile([1, 2 * 1024], F32, tag="d1", name="dr")
        nc.sync.dma_start(dr[:], rs[D:D + 1, :])
        sp = p2nrm.tile([P, 32], F32, tag="sp", name="sp")
        nc.sync.dma_start(
            sp[0:D, :], dr[:].rearrange("a (p f) -> (a p) f", f=32))
        rp = p2nrm.tile([P, 32], F32, tag="rp", name="rp")
        nc.vector.reciprocal(rp[0:D, :], sp[0:D, :])
        dr2 = p2dram.tile([1, 2 * 1024], F32, tag="d2", name="dr2")
        nc.sync.dma_start(
            dr2[:].rearrange("a (p f) -> (a p) f", f=32), rp[0:D, :])
        return (dr2,)

    def emit_stage(pv1, pv2):
        # stage PV out of PSUM right after the pair's last PV matmuls so
        # the pv slots free up for the next pair
        st = p2nrm.tile([P, 1024], BF16, tag="st", name="st")
        nc.vector.tensor_copy(st[0:D, :], pv1[0:D, :])
        nc.vector.tensor_copy(st[D:P, :], pv2[0:D, :])
        return st

    def emit_norm_mul(qh, p, st, dr2):
        # broadcast the reciprocal rowsums across partitions, normalize
        qbase = p * S + qh * QH
        rb = p2nrm.tile([P, 1024], F32, tag="rb", name="rb")
        nc.sync.dma_start(rb[0:D, :], dr2[:, 0:1024].partition_broadcast(D))
        nc.sync.dma_start(rb[D:P, :], dr2[:, 1024:2048].partition_broadcast(D))
        nc.vector.tensor_mul(
            otn[0:D, qbase:qbase + QH], st[0:D, :], rb[0:D, :]
        )
        nc.vector.tensor_mul(
            otn[D:P, qbase:qbase + QH], st[D:P, :], rb[D:P, :]
        )

    def emit_y_group(qi, ec):
        # Y[qi*128:(qi+1)*128, ec*512:(ec+1)*512] = sum_p otn_p^T @ wo_p
        yps = ps_sc.tile([P, 512], F32, tag="sc", name="yps")
        for p in range(ND):
            nc.tensor.matmul(
                yps[:],
                otn[:, p * S + qi * P: p * S + (qi + 1) * P],
                wo[:, p * E + ec * 512: p * E + ec * 512 + 512],
                start=(p == 0), stop=(p == ND - 1),
            )
        ysb = p3sb.tile([P, 512], F32, tag="ys", name="ysb")
        nc.vector.tensor_copy(ysb[:], yps[:])
        nc.sync.dma_start(
            d["Yp"].ap()[qi * P:(qi + 1) * P, ec * 512:(ec + 1) * 512],
            ysb[:],
        )

    # ---------------- flat step schedule ----------------
    # Projection chunk queue, deadline-ordered (popped 2 per 3 steps):
    # dd0's remaining K chunks first (k-tiles 4+ of pair 0), then each
    # later dd's K + q-half-0 Q chunks before its pair starts, and all
    # q-half-1 Q chunks last (needed only from step 64).
    def kq(dd, w_sb, out_t, b_t, sc):
        return lambda: emit_kq_chunk(dd, w_sb, out_t, b_t, sc)

    p1q = []
    for dd in range(1, ND):
        p1q += [kq(dd, wk, kt, bk, sc) for sc in range(4)]
        p1q += [kq(dd, wq, qt, bq, sc) for sc in range(4)]

    # phase-3 (qi, ec) groups of q-half 0, interleaved into q-half 1
    p3q0 = [(qi, ec) for qi in range(NS // 2) for ec in range(2)]

    # prologue: dd0 projections + first two V k-tiles + first masks
    for k in range(2):
        emit_mt_dma(k, 0)
    for sc in range(4):
        emit_kq_chunk(0, wk, kt, bk, sc)
    for sc in range(4):
        emit_kq_chunk(0, wq, qt, bq, sc)
    emit_v(0)
    emit_v(1)

    prev = None       # (qh, p, k) whose exp/mask + PV are pending
    pend_norm = None  # (qh, p, pv1, pv2, dsc1, dsc2): norm tail pending

    for step in range(2 * ND * NK):
        qh, rem = divmod(step, ND * NK)
        p, k = divmod(rem, NK)

        # 1. exp+mask of the previous k-tile — emitted before anything
        #    reuses its "sc"/"es"/"pr" slots.
        if prev is not None:
            emit_em(*prev)

        # 2. interleaved fill work (allocates "sc" slots; safe now)
        if step < NS - 2:
            emit_v(step + 2)          # V k-tile, two ahead of PV's use
        if step % 2 == 1 and p1q:
            p1q.pop(0)()              # QT/KT projection chunk
        if qh == 0 and p == 0 and k + 2 < NK:
            emit_mt_dma(k + 2, 0)
        if qh == 1 and p == 0:
            if k == 0:
                emit_mt_dma(0, 1)
            if k + 1 < NK:
                emit_mt_dma(k + 1, 1)
        if qh == 1 and p3q0 and ((k % 3 == 0 and k > 0) or (p == 2 and k == 14)):
            if p < 3:
                emit_y_group(*p3q0.pop(0))

        # 3. deferred normalize-multiply of the previous pair (the
        #    reciprocal DMA chain has had ~2 steps to land)
        if pend_norm is not None and k == 2:
            emit_norm_mul(*pend_norm)
            pend_norm = None

        # 4. this k-tile's scores
        emit_scores(qh, p, k)

        # 5. previous k-tile's PV accumulate (+ recips after a pair's
        #    final PV)
        if prev is not None:
            pv1, pv2 = emit_pv(*prev)
            if prev[2] == NK - 1:
                (dr2,) = emit_recips(pv1, pv2)
                stt = emit_stage(pv1, pv2)
                pend_norm = prev[:2] + (stt, dr2)
        prev = (qh, p, k)

    # drain: last k-tile, then the last pair's normalization with an
    # on-chip PE broadcast of the reciprocal rowsums (the DMA round trip
    # would sit on the critical path here), then phase-3 q-half-1.
    emit_em(*prev)
    opv1, opv2 = emit_pv(*prev)
    oqh, op = prev[0], prev[1]
    if pend_norm is not None:
        emit_norm_mul(*pend_norm)
        pend_norm = None
    (dr2,) = emit_recips(opv1, opv2)
    stt = emit_stage(opv1, opv2)
    emit_norm_mul(oqh, op, stt, dr2)
    assert not p1q and not p3q0, (len(p1q), len(p3q0))

    for qi in range(NS // 2, NS):
        for ec in range(2):
            emit_y_group(qi, ec)


def build_module(reps=1):
    from contextlib import ExitStack

    nc = bacc.Bacc("TRN2", target_bir_lowering=False, debug=False)
    d = {
        "XT": nc.dram_tensor("XT", [E, S], BF16, kind="ExternalInput"),
        "maskT": nc.dram_tensor("maskT", [S, S], BF16, kind="ExternalInput"),
        "wQ": nc.dram_tensor("wQ", [E, DH], BF16, kind="ExternalInput"),
        "wK": nc.dram_tensor("wK", [E, DH], BF16, kind="ExternalInput"),
        "wV": nc.dram_tensor("wV", [E, DH], BF16, kind="ExternalInput"),
        "wO": nc.dram_tensor("wO", [DH, E], BF16, kind="ExternalInput"),
        "bQ": nc.dram_tensor("bQ", [DH], F32, kind="ExternalInput"),
        "bK": nc.dram_tensor("bK", [DH], F32, kind="ExternalInput"),
        "bV": nc.dram_tensor("bV", [DH], F32, kind="ExternalInput"),
        "Yp": nc.dram_tensor("Yp", [S, E], F32, kind="ExternalOutput"),
    }
    with tile.TileContext(nc) as tc:
        for _ in range(reps):
            with ExitStack() as ctx:
                _emit(nc, tc, ctx, d)
    nc.compile()
    return nc


def make_in_maps(X, mask, wQ, bQ, wK, bK, wV, bV, wO, bO):
    """Per-core input dicts. Core c: batch c//2, head-half c%2."""
    in_maps = []
    for c in range(8):
        b, hh = c // 2, c % 2
        cols = slice(hh * DH, (hh + 1) * DH)
        in_maps.append({
            "XT": np.ascontiguousarray(np.asarray(X[b]).T).astype(ml_dtypes.bfloat16),
            "maskT": np.ascontiguousarray(
                np.asarray(mask[b, 0]).T
            ).astype(ml_dtypes.bfloat16),
            "wQ": (np.asarray(wQ[:, cols]) * np.float32(0.125)).astype(ml_dtypes.bfloat16),
            "wK": np.asarray(wK[:, cols]).astype(ml_dtypes.bfloat16),
            "wV": np.asarray(wV[:, cols]).astype(ml_dtypes.bfloat16),
            "wO": np.asarray(wO[cols, :]).astype(ml_dtypes.bfloat16),
            "bQ": np.ascontiguousarray(np.asarray(bQ[cols])) * np.float32(0.125),
            "bK": np.ascontiguousarray(np.asarray(bK[cols])),
            "bV": np.ascontiguousarray(np.asarray(bV[cols])),
        })
    return in_maps


_NC = None


def kernel(X, mask, wQ, bQ, wK, bK, wV, bV, wO, bO):
    global _NC
    if _NC is None:
        _NC = build_module()
    in_maps = make_in_maps(X, mask, wQ, bQ, wK, bK, wV, bV, wO, bO)
    res = run_bass_kernel_spmd(_NC, in_maps, list(range(8)))
    B = 4
    Y = np.empty((B, S, E), dtype=np.float32)
    bO = np.asarray(bO, dtype=np.float32)
    for b in range(B):
        Y[b] = res.results[2 * b]["Yp"] + res.results[2 * b + 1]["Yp"] + bO
    return Y
